# revision 37
# baseline (speedup 1.0000x reference)
"""Trainium2 Bass kernel for nn_CriticUAVob (attention-pool critic).

Math per item b (4096 total): two attention-pool branches over s_b [N=128, 3]
followed by a small MLP.  With s' = [s, 1] (N x 4) and A = Wq' Wk'^T / 4:

    S = s' A s'^T,  U = exp(S),  Z[n] = sum_m U[n,m]
    pooled = (1/N) * sum_n (U[n,:] / Z[n]) @ V,   V = s' Wv'
           = (1/N) * t^T Wv',   t[k] = sum_n (sum_m U[n,m] s'[m,k]) / Z[n]

Device pipeline (per quad of 4 items, batch data-parallel over 8 cores):
  - one DMA brings Y = A^T s'^T per item in a block-diagonal layout (qt) plus
    s'^T stacked (sst); a second DMA brings zero-padded s' variants (snatp)
  - 2 matmuls (lhsT=sst[16,128], rhs=qt[16,512]) -> X = S^T per item, both
    branches; the block-diagonal qt kills cross-item terms
  - 2 ScalarE exp -> U^T in bf16
  - 4 accumulating matmuls (lhsT = zero-padded s'_i [128,16], rhs = U^T item
    cols) -> G[(i,k),(b,n)] with no cross-item garbage; s' ones-column makes
    row (i,3) = Z
  - tiny const matmul replicates each Z row over its item's 4 rows, then
    vector fast-reciprocal, gpsimd multiply, vector reduce produce t[(i,k)]
    per branch straight into an accumulator tile
  - batched MLP over all 512 items at the end (first layer reads tbig's
    packed layout directly via zero-padded weight variants; the resulting
    item permutation is undone for free in the output DMA)

All PE inputs are bf16 (4x matmul throughput vs fp32, half the LDWEIGHTS).
The loop is software-pipelined with per-stage lags so each engine's stream
depends only on >=1-iteration-old work; emission interleaves st and G
matmuls so consecutive PE ops hit different PSUM banks.
"""
import os
import sys
import numpy as np
import ml_dtypes

sys.path.insert(0, "/opt/trn_rl_repo")

import concourse.bass as bass
import concourse.tile as tile
from concourse import bacc, mybir
from concourse import bass_utils

N_CORES = 8
B = 4096
N = 128
BC = B // N_CORES          # 512 items per core
QUADS = BC // 4            # 128 groups of 4 items
F32 = mybir.dt.float32
BF16 = mybir.dt.bfloat16
AF = mybir.ActivationFunctionType
ALU = mybir.AluOpType
BF = ml_dtypes.bfloat16

_cache = {}


def _build():
    nc = bacc.Bacc(
        "TRN2",
        target_bir_lowering=False,
        debug=False,
        enable_asserts=False,
        num_devices=N_CORES,
    )
    # per-quad data: qt [16, 1024] block-diag A^T s'^T (both branches) then
    # sst [16, 128] = stacked s'^T, packed in one contiguous record
    qtss_t = nc.dram_tensor("qtss", [QUADS, 16, 1152], BF16, kind="ExternalInput")
    # zero-padded s' variants: cols (i, k16); variant i nonzero only in 4i..4i+4
    snatp_t = nc.dram_tensor("snatp", [QUADS, 128, 64], BF16, kind="ExternalInput")
    rep16_t = nc.dram_tensor("rep16", [16, 16], BF16, kind="ExternalInput")
    wcih_t = nc.dram_tensor("wcih", [16, 512], BF16, kind="ExternalInput")
    w1_t = nc.dram_tensor("w1", [64, 128], BF16, kind="ExternalInput")
    w2_t = nc.dram_tensor("w2", [128, 128], BF16, kind="ExternalInput")
    w3_t = nc.dram_tensor("w3", [128, 1], BF16, kind="ExternalInput")
    b1_t = nc.dram_tensor("b1", [128, 1], F32, kind="ExternalInput")
    b2_t = nc.dram_tensor("b2", [128, 1], F32, kind="ExternalInput")
    b3_t = nc.dram_tensor("b3rep", [1, BC], F32, kind="ExternalInput")
    out_t = nc.dram_tensor("out", [BC, 1], F32, kind="ExternalOutput")

    qtss_ap = qtss_t.ap()
    snatp_ap = snatp_t.ap()

    with tile.TileContext(nc) as tc:
        with (
            tc.tile_pool(name="singles", bufs=1) as singles,
            tc.tile_pool(name="qsb", bufs=4) as qsb,
            tc.tile_pool(name="pst", bufs=3, space="PSUM") as pst,
            tc.tile_pool(name="psmall", bufs=2, space="PSUM") as psmall,
        ):
            rep16 = singles.tile([16, 16], BF16)
            nc.scalar.dma_start(rep16[:], rep16_t.ap())
            wcih = singles.tile([16, 512], BF16)
            nc.scalar.dma_start(wcih[:], wcih_t.ap())
            w1 = singles.tile([64, 128], BF16)
            nc.scalar.dma_start(w1[:], w1_t.ap())
            w2 = singles.tile([128, 128], BF16)
            nc.scalar.dma_start(w2[:], w2_t.ap())
            w3 = singles.tile([128, 1], BF16)
            nc.scalar.dma_start(w3[:], w3_t.ap())
            b1 = singles.tile([128, 1], F32)
            nc.scalar.dma_start(b1[:], b1_t.ap())
            b2 = singles.tile([128, 1], F32)
            nc.scalar.dma_start(b2[:], b2_t.ap())
            b3r = singles.tile([1, BC], F32)
            nc.scalar.dma_start(b3r[:], b3_t.ap())
            # t accumulator: rows (i,k), cols (quad, branch)
            tbig = singles.tile([16, 2 * QUADS], F32)

            # Software pipeline: stage lags keep every engine's next
            # instruction dependent only on work from >=1 iteration ago, so
            # the PE never stalls mid-stream (and can ramp to full clock).
            qtssT, snatpT, ps_stT, utT, psgzT, g_sbT, r_sbT, pgT = (
                {}, {}, {}, {}, {}, {}, {}, {},
            )
            L_DMA, L_ST, L_G, L_Z, L_T = 0, 1, 2, 3, 4

            def live(j, lag):
                return 0 <= j - lag < QUADS

            for j in range(QUADS + L_T + 1):
                if live(j, L_DMA):
                    q = j
                    qtssT[q] = qsb.tile([16, 1152], BF16, tag="qtss", name="qtss")
                    nc.sync.dma_start(qtssT[q][:], qtss_ap[q])
                    snatpT[q] = qsb.tile([128, 64], BF16, tag="snatp", name="snatp")
                    nc.gpsimd.dma_start(snatpT[q][:], snatp_ap[q])

                # PE emission interleaves the st pair (quad q, pst bank) with
                # the G group (quad q-1, psmall bank) so consecutive matmuls
                # hit different PSUM banks and their drain phases overlap.
                gq = j - L_G
                if live(j, L_G):
                    psgzT[gq] = psmall.tile([16, 512], F32, tag="psgz",
                                            name="psgz")

                def g_mm(i):
                    nc.tensor.matmul(
                        psgzT[gq][:, 0:256],
                        snatpT[gq][:, 16 * i:16 * (i + 1)],
                        utT[gq][:, 256 * i:256 * (i + 1)],
                        start=(i == 0),
                        stop=(i == 3),
                    )

                if live(j, L_ST):
                    # X = S^T per item (key idx on partitions), both branches
                    q = j - L_ST
                    qtss = qtssT[q]
                    sst = qtss[:, 1024:1152]
                    ps_st = pst.tile([128, 1024], F32, tag="st", name="ps_st")
                    ps_stT[q] = ps_st
                    nc.tensor.matmul(ps_st[:, 0:512], sst, qtss[:, 0:512])
                    if live(j, L_G):
                        g_mm(0)
                        g_mm(1)
                    nc.tensor.matmul(ps_st[:, 512:1024], sst, qtss[:, 512:1024])
                    if live(j, L_G):
                        g_mm(2)
                        g_mm(3)
                    # U^T = exp(X), bf16; stored as cols (i, b, n) so each
                    # item's G-matmul rhs is a contiguous 2D slice
                    ut = qsb.tile([128, 1024], BF16, tag="ut", name="ut")
                    utT[q] = ut
                    ut_v = ut[:].rearrange("m (i b n) -> m b i n", i=4, b=2)
                    ps_st_v = ps_st[:].rearrange("m (b i n) -> m b i n", b=2, i=4)
                    nc.scalar.activation(ut_v, ps_st_v, AF.Exp)
                    del qtssT[q]
                elif live(j, L_G):
                    for i in range(4):
                        g_mm(i)

                if live(j, L_G):
                    # G[(i,k),(b,n)] = sum_m s'_i[m,k] U^T[m,(b,n)]; (i,3)=Z
                    g_sb = qsb.tile([16, 256], BF16, tag="g_sb", name="g_sb")
                    g_sbT[gq] = g_sb
                    nc.vector.tensor_copy(g_sb[:], psgzT[gq][:, 0:256])
                    del ps_stT[gq], snatpT[gq]

                if live(j, L_Z):
                    # replicate each item's Z row over its 4 rows, then 1/Z
                    q = j - L_Z
                    psgz = psgzT[q]
                    nc.tensor.matmul(psgz[:, 256:512], rep16[:], g_sbT[q][:])
                    r_sb = qsb.tile([16, 256], F32, tag="r_sb", name="r_sb")
                    r_sbT[q] = r_sb
                    nc.vector.reciprocal_approx_fast(r_sb[:], psgz[:, 256:512])
                    del utT[q]

                if live(j, L_T):
                    # t[(i,k), (q,b)] = sum_n G * (1/Z)
                    q = j - L_T
                    pg = qsb.tile([16, 256], F32, tag="pg", name="pg")
                    nc.gpsimd.tensor_mul(pg[:], g_sbT[q][:], r_sbT[q][:])
                    pg3 = pg[:].rearrange("p (b n) -> p b n", b=2)
                    nc.vector.tensor_reduce(
                        tbig[:, 2 * q:2 * (q + 1)], pg3,
                        axis=mybir.AxisListType.X, op=ALU.add,
                    )
                    del psgzT[q], g_sbT[q], r_sbT[q]

            # First MLP layer directly from tbig: 8 matmuls with zero-padded
            # weight variants (wcih) pick each item-in-quad's rows out of
            # tbig's [(i,k), (q,b)] layout; items land as cols (i, q) and the
            # permutation is undone for free in the final output DMA.
            tbig_bf = singles.tile([16, 2 * QUADS], BF16)
            nc.vector.tensor_copy(tbig_bf[:], tbig[:])
            tb3 = tbig_bf[:].rearrange("p (q b) -> p b q", b=2)
            ps_h = pst.tile([64, BC], F32, tag="st")
            for i in range(4):
                for b in range(2):
                    nc.tensor.matmul(
                        ps_h[:, 128 * i:128 * (i + 1)],
                        wcih[:, 64 * (2 * i + b):64 * (2 * i + b + 1)],
                        tb3[:, b, :],
                        start=(b == 0),
                        stop=(b == 1),
                    )
            h_sb = singles.tile([64, BC], BF16)
            nc.vector.tensor_copy(h_sb[:], ps_h[:])

            ps_z1 = pst.tile([128, BC], F32, tag="st")
            nc.tensor.matmul(ps_z1[:], w1[:], h_sb[:])
            h1 = singles.tile([128, BC], BF16)
            nc.scalar.activation(h1[:], ps_z1[:], AF.Tanh, bias=b1[:])

            ps_z2 = pst.tile([128, BC], F32, tag="st")
            nc.tensor.matmul(ps_z2[:], w2[:], h1[:])
            h2 = singles.tile([128, BC], BF16)
            nc.scalar.activation(h2[:], ps_z2[:], AF.Tanh, bias=b2[:])

            ps_z3 = psmall.tile([1, BC], F32, tag="psgz")
            nc.tensor.matmul(ps_z3[:], w3[:], h2[:])
            y_sb = singles.tile([1, BC], F32)
            nc.vector.tensor_add(y_sb[:], ps_z3[:], b3r[:])

            nc.sync.dma_start(
                out_t.ap().rearrange("(q i) o -> o i q", i=4),
                y_sb[:].rearrange("o (i q) -> o i q", i=4),
            )

    nc.compile()
    return nc


def _host_prep(inputs):
    f = lambda x: np.asarray(x, dtype=np.float32)
    s_obs = f(inputs["s_obs"])

    def aug_w(W, b):
        return np.vstack([f(W), f(b).reshape(1, -1)])  # [4, dout]

    Wq_rs = aug_w(inputs["Wq_rs"], inputs["bq_rs"])
    Wk_rs = aug_w(inputs["Wk_rs"], inputs["bk_rs"])
    Wv_rs = aug_w(inputs["Wv_rs"], inputs["bv_rs"])
    Wq_tg = aug_w(inputs["Wq_tg"], inputs["bq_tg"])
    Wk_tg = aug_w(inputs["Wk_tg"], inputs["bk_tg"])
    Wv_tg = aug_w(inputs["Wv_tg"], inputs["bv_tg"])

    scale = 1.0 / np.sqrt(16.0)
    A_rs = (Wq_rs @ Wk_rs.T * scale).astype(np.float32)   # [4, 4]
    A_tg = (Wq_tg @ Wk_tg.T * scale).astype(np.float32)

    ones = np.ones((B, N, 1), np.float32)
    s_aug = np.concatenate([s_obs, ones], axis=2)          # [B, 128, 4]

    # Y_b[item] = A_b^T s'^T : [B, 4, 128]
    Y = np.stack([
        np.einsum("kj,ink->ijn", A_rs, s_aug),
        np.einsum("kj,ink->ijn", A_tg, s_aug),
    ], axis=0).astype(np.float32)                          # [2, B, 4, 128]

    rep16 = np.zeros((16, 16), BF)
    for i in range(4):
        rep16[4 * i + 3, 4 * i:4 * (i + 1)] = 1.0

    wcrs = np.zeros((4, 64), np.float32)
    wctg = np.zeros((4, 64), np.float32)
    wcrs[:, 0:32] = Wv_rs / N
    wctg[:, 32:64] = Wv_tg / N
    # zero-padded variants: wcih[(i,k), (2i+b)*64 + d] = wc_b[k, d]
    wcih = np.zeros((16, 512), np.float32)
    for i in range(4):
        wcih[4 * i:4 * (i + 1), 64 * (2 * i):64 * (2 * i + 1)] = wcrs
        wcih[4 * i:4 * (i + 1), 64 * (2 * i + 1):64 * (2 * i + 2)] = wctg

    w1 = f(inputs["W1"])                       # [64, 128]
    b1 = f(inputs["b1"]).reshape(128, 1)
    w2 = f(inputs["W2"])                       # [128, 128]
    b2 = f(inputs["b2"]).reshape(128, 1)
    w3 = f(inputs["W3"])                       # [128, 1]
    b3rep = np.full((1, BC), float(np.asarray(inputs["b3"]).reshape(-1)[0]),
                    np.float32)

    common = dict(
        rep16=rep16,
        wcih=wcih.astype(BF),
        w1=w1.astype(BF), w2=w2.astype(BF), w3=w3.astype(BF),
        b1=b1, b2=b2, b3rep=b3rep,
    )

    in_maps = []
    for c in range(N_CORES):
        lo, hi = c * BC, (c + 1) * BC
        sa = s_aug[lo:hi].reshape(QUADS, 4, N, 4)          # [Q, i, n, k]
        Yc = Y[:, lo:hi].reshape(2, QUADS, 4, 4, N)        # [b, Q, i, j, n]

        # qt [Q, (i,j)=16, (b,i',n)=1024], block-diagonal in (i, i')
        qt = np.zeros((QUADS, 4, 4, 2, 4, N), np.float32)  # q i j b i' n
        for i in range(4):
            qt[:, i, :, 0, i, :] = Yc[0, :, i]
            qt[:, i, :, 1, i, :] = Yc[1, :, i]
        qt = qt.reshape(QUADS, 16, 1024)

        # sst [Q, (i,k)=16, n=128]
        sst = sa.transpose(0, 1, 3, 2).reshape(QUADS, 16, N)

        qtss = np.concatenate([qt, sst], axis=2).astype(BF)  # [Q, 16, 1152]

        # snatp [Q, 128, (i, k16)=64]: variant i nonzero only in cols 4i..4i+4
        snatp = np.zeros((QUADS, N, 4, 16), np.float32)
        for i in range(4):
            snatp[:, :, i, 4 * i:4 * (i + 1)] = sa[:, i]
        snatp = snatp.reshape(QUADS, N, 64).astype(BF)

        m = dict(common)
        m["qtss"] = np.ascontiguousarray(qtss)
        m["snatp"] = np.ascontiguousarray(snatp)
        in_maps.append(m)
    return in_maps


def kernel(**inputs):
    if "nc" not in _cache:
        _cache["nc"] = _build()
    nc = _cache["nc"]
    in_maps = _host_prep(inputs)
    trace = os.environ.get("KERNEL_TRACE", "0") == "1"
    res = bass_utils.run_bass_kernel_spmd(
        nc, in_maps, core_ids=list(range(N_CORES)), trace=trace
    )
    _cache["last"] = res
    out = np.concatenate([r["out"] for r in res.results], axis=0)
    return out.astype(np.float32)
